# revision 4
# baseline (speedup 1.0000x reference)
import sys

if '/opt/trn_rl_repo' not in sys.path:
    sys.path.insert(0, '/opt/trn_rl_repo')

import numpy as np

import concourse.bass as bass
import concourse.bacc as bacc
import concourse.mybir as mybir
from concourse.tile import TileContext
from concourse.bass_utils import run_bass_kernel_spmd

# Problem constants (hardcoded per harness contract)
M, N, ITERS = 63, 127, 8
MH = 64            # m padded to 64 for clean power-of-2 tree sums
P, J = 128, 2      # 128 partitions x 2 batch rows per partition = 256 batch/core
CORES = 8
BL = P * J         # batch per core
CH = 8             # m-chunk size (MH = 8 chunks of 8)
NCHUNK = MH // CH
BIGC = 1e30
L_WEIGHT = 0.5
SCALAR_FACTOR = 0.01

F32 = mybir.dt.float32
BF16 = mybir.dt.bfloat16
ALU = mybir.AluOpType
ACT = mybir.ActivationFunctionType
AX = mybir.AxisListType

_NC_CACHE = {}


def _bcast_dram(ap, parts=P):
    """Partition-broadcast read AP for a DRAM tensor (step 0 over partitions)."""
    return bass.AP(tensor=ap.tensor, offset=0,
                   ap=[[0, parts]] + [list(d) for d in ap.ap])


def _build_nc():
    nc = bacc.Bacc(None, target_bir_lowering=False)

    soft_d = nc.dram_tensor("soft", [P, J, N], F32, kind="ExternalInput")
    hf_d = nc.dram_tensor("hf", [MH, N], F32, kind="ExternalInput")
    bigoff_d = nc.dram_tensor("bigoff", [MH, N], F32, kind="ExternalInput")
    c1_d = nc.dram_tensor("c1", [MH], F32, kind="ExternalInput")
    wp_d = nc.dram_tensor("wp", [1], F32, kind="ExternalInput")
    out_d = nc.dram_tensor("out", [ITERS + 1, P, J, N], F32, kind="ExternalOutput")

    with TileContext(nc) as tc:
        with (
            tc.tile_pool(name="const", bufs=1) as cpool,
            tc.tile_pool(name="state", bufs=1) as spool,
            tc.tile_pool(name="tpool", bufs=2) as tpool,
            tc.tile_pool(name="work", bufs=1) as wpool,
            tc.tile_pool(name="tree", bufs=1) as rpool,
            tc.tile_pool(name="small", bufs=1) as mpool,
        ):
            # ---- constants ----
            hf = cpool.tile([P, MH, N], BF16, tag="hf")
            bigoff = cpool.tile([P, MH, N], BF16, tag="bigoff")
            c1 = cpool.tile([P, MH], F32, tag="c1")
            wp = cpool.tile([P, 1], F32, tag="wp")
            wm8 = cpool.tile([P, 1], F32, tag="wm8")
            soft = cpool.tile([P, J, N], F32, tag="soft")

            nc.gpsimd.dma_start(out=hf, in_=_bcast_dram(hf_d[:]))
            nc.gpsimd.dma_start(out=bigoff, in_=_bcast_dram(bigoff_d[:]))
            nc.gpsimd.dma_start(out=c1, in_=_bcast_dram(c1_d[:]))
            nc.gpsimd.dma_start(out=wp, in_=_bcast_dram(wp_d[:]))
            nc.gpsimd.dma_start(out=soft, in_=soft_d[:])
            nc.vector.tensor_scalar(out=wm8, in0=wp, scalar1=-8.0, scalar2=None,
                                    op0=ALU.mult)

            # ---- state ----
            cv = spool.tile([P, J, MH, N], F32, tag="cv")
            EQ = spool.tile([P, J, MH, N], BF16, tag="EQ")
            SMH = spool.tile([P, J, MH, N], BF16, tag="SMH")
            MIN1 = spool.tile([P, J, MH], F32, tag="MIN1")
            MIN2 = spool.tile([P, J, MH], F32, tag="MIN2")
            CNT = spool.tile([P, J, MH], F32, tag="CNT")
            SS = spool.tile([P, J, MH], F32, tag="SS")

            nc.gpsimd.memset(cv[:], 0.0)

            # T(0) = soft;  outs[0] = soft
            T = tpool.tile([P, J, N], F32, tag="T")
            nc.vector.tensor_copy(T[:], soft[:])
            nc.sync.dma_start(out=out_d[:][0], in_=soft)

            for it in range(ITERS):
                # ---------- pass 1: vm-derived quantities per m-chunk ----------
                for c in range(NCHUNK):
                    c0, c1e = c * CH, (c + 1) * CH
                    cvc = cv[:, :, c0:c1e, :]
                    T_b = T[:, :, None, :].broadcast_to([P, J, CH, N])
                    bo_v = bigoff[:, None, c0:c1e, :].broadcast_to([P, J, CH, N])
                    hf_v = hf[:, None, c0:c1e, :].broadcast_to([P, J, CH, N])

                    t0 = wpool.tile([P, J, CH, N], F32, tag="A")
                    nc.vector.tensor_tensor(out=t0, in0=T_b, in1=cvc, op=ALU.subtract)
                    vm = wpool.tile([P, J, CH, N], F32, tag="B")
                    nc.vector.scalar_tensor_tensor(out=vm, in0=bo_v, scalar=-1.0,
                                                   in1=t0, op0=ALU.mult, op1=ALU.add)
                    # min1 = min |vm| (DVE, fused abs)
                    nc.vector.tensor_reduce(out=MIN1[:, :, c0:c1e], in_=vm, axis=AX.X,
                                            op=ALU.min, apply_absolute_value=True)
                    # a = |vm| (ScalarE), smr = sign(vm) (ScalarE)
                    a = wpool.tile([P, J, CH, N], F32, tag="A")
                    nc.scalar.activation(out=a, in_=vm, func=ACT.Abs)
                    smr = wpool.tile([P, J, CH, N], BF16, tag="C")
                    nc.scalar.activation(out=smr, in_=vm, func=ACT.Sign)
                    # smh = sign * mask (bf16 2x)
                    nc.vector.tensor_tensor(out=SMH[:, :, c0:c1e, :], in0=smr,
                                            in1=hf_v, op=ALU.mult)
                    # S = sum(smh) over n
                    nc.vector.tensor_reduce(out=SS[:, :, c0:c1e],
                                            in_=SMH[:, :, c0:c1e, :], axis=AX.X,
                                            op=ALU.add)
                    # eq = (a == min1)
                    m1_b = MIN1[:, :, c0:c1e, None].broadcast_to([P, J, CH, N])
                    nc.vector.tensor_tensor(out=EQ[:, :, c0:c1e, :], in0=a, in1=m1_b,
                                            op=ALU.is_equal)
                    # knockout and min2
                    k = wpool.tile([P, J, CH, N], F32, tag="B")
                    nc.vector.scalar_tensor_tensor(out=k, in0=EQ[:, :, c0:c1e, :],
                                                   scalar=BIGC, in1=a,
                                                   op0=ALU.mult, op1=ALU.add)
                    nc.vector.tensor_reduce(out=MIN2[:, :, c0:c1e], in_=k, axis=AX.X,
                                            op=ALU.min)
                    # tie count
                    nc.vector.tensor_reduce(out=CNT[:, :, c0:c1e],
                                            in_=EQ[:, :, c0:c1e, :], axis=AX.X,
                                            op=ALU.add)

                # ---------- per-iteration small ops on [P, J, MH] ----------
                c1_v = c1[:, None, :].broadcast_to([P, J, MH])
                nn = mpool.tile([P, J, MH], F32, tag="nn")
                nc.vector.scalar_tensor_tensor(out=nn, in0=SS, scalar=-0.5, in1=c1_v,
                                               op0=ALU.mult, op1=ALU.add)
                t1r = mpool.tile([P, J, MH], F32, tag="t1r")
                nc.vector.tensor_scalar(out=t1r, in0=nn, scalar1=0.5,
                                        scalar2=float(2 ** 23),
                                        op0=ALU.mult, op1=ALU.add)
                t2r = mpool.tile([P, J, MH], F32, tag="t2r")
                nc.vector.tensor_scalar(out=t2r, in0=t1r, scalar1=float(2 ** 23),
                                        scalar2=None, op0=ALU.subtract)
                fr = mpool.tile([P, J, MH], F32, tag="fr")
                nc.vector.scalar_tensor_tensor(out=fr, in0=nn, scalar=0.5, in1=t2r,
                                               op0=ALU.mult, op1=ALU.subtract)
                p4 = mpool.tile([P, J, MH], F32, tag="p4")
                nc.vector.tensor_tensor(out=p4, in0=fr, in1=fr, op=ALU.mult)
                pw = mpool.tile([P, J, MH], F32, tag="pw")
                nc.vector.tensor_scalar(out=pw, in0=p4, scalar1=wm8, scalar2=wp,
                                        op0=ALU.mult, op1=ALU.add)
                # tie fix: min2e = min2 + (cnt>=2)*(min1-min2)
                tie = mpool.tile([P, J, MH], F32, tag="tie")
                nc.vector.tensor_scalar(out=tie, in0=CNT, scalar1=1.5, scalar2=None,
                                        op0=ALU.is_ge)
                dmm = mpool.tile([P, J, MH], F32, tag="dmm")
                nc.vector.tensor_tensor(out=dmm, in0=MIN1, in1=MIN2, op=ALU.subtract)
                tcor = mpool.tile([P, J, MH], F32, tag="tcor")
                nc.vector.tensor_tensor(out=tcor, in0=tie, in1=dmm, op=ALU.mult)
                min2e = mpool.tile([P, J, MH], F32, tag="min2e")
                nc.vector.tensor_tensor(out=min2e, in0=MIN2, in1=tcor, op=ALU.add)
                # Pm = pw*min1 ; Qm = pw*(min2e-min1)
                Pm = mpool.tile([P, J, MH], F32, tag="Pm")
                nc.vector.tensor_tensor(out=Pm, in0=pw, in1=MIN1, op=ALU.mult)
                d21 = mpool.tile([P, J, MH], F32, tag="d21")
                nc.vector.tensor_tensor(out=d21, in0=min2e, in1=MIN1, op=ALU.subtract)
                Qm = mpool.tile([P, J, MH], F32, tag="Qm")
                nc.vector.tensor_tensor(out=Qm, in0=pw, in1=d21, op=ALU.mult)

                # ---------- pass 2: cv update + tree-sum ----------
                Tacc = tpool.tile([P, J, N], F32, tag="Tacc")
                for c in range(NCHUNK):
                    c0, c1e = c * CH, (c + 1) * CH
                    Qm_b = Qm[:, :, c0:c1e, None].broadcast_to([P, J, CH, N])
                    Pm_b = Pm[:, :, c0:c1e, None].broadcast_to([P, J, CH, N])

                    u = wpool.tile([P, J, CH, N], F32, tag="A")
                    nc.vector.tensor_tensor(out=u, in0=EQ[:, :, c0:c1e, :], in1=Qm_b,
                                            op=ALU.mult)
                    v = wpool.tile([P, J, CH, N], F32, tag="B")
                    nc.vector.tensor_tensor(out=v, in0=u, in1=Pm_b, op=ALU.add)
                    nc.vector.tensor_tensor(out=cv[:, :, c0:c1e, :], in0=v,
                                            in1=SMH[:, :, c0:c1e, :], op=ALU.mult)
                    # tree-sum this chunk of new cv into Tacc
                    s1 = rpool.tile([P, J, 4, N], F32, tag="s1")
                    nc.vector.tensor_tensor(out=s1, in0=cv[:, :, c0:c0 + 4, :],
                                            in1=cv[:, :, c0 + 4:c1e, :], op=ALU.add)
                    s2 = rpool.tile([P, J, 2, N], F32, tag="s2")
                    nc.vector.tensor_tensor(out=s2, in0=s1[:, :, 0:2, :],
                                            in1=s1[:, :, 2:4, :], op=ALU.add)
                    if c == 0:
                        nc.vector.tensor_tensor(out=Tacc, in0=s2[:, :, 0, :],
                                                in1=s2[:, :, 1, :], op=ALU.add)
                    else:
                        s3 = rpool.tile([P, J, N], F32, tag="s3")
                        nc.vector.tensor_tensor(out=s3, in0=s2[:, :, 0, :],
                                                in1=s2[:, :, 1, :], op=ALU.add)
                        nc.vector.tensor_tensor(out=Tacc, in0=Tacc, in1=s3,
                                                op=ALU.add)

                # T(it+1) = Tacc + soft ; DMA out
                T = tpool.tile([P, J, N], F32, tag="T")
                nc.vector.tensor_tensor(out=T, in0=Tacc, in1=soft, op=ALU.add)
                nc.sync.dma_start(out=out_d[:][it + 1], in_=T)

    nc.finalize()
    return nc


def _get_runner():
    """Build (once) a persistent jitted 8-core runner for the bass kernel.

    Replicates concourse.bass2jax.run_bass_via_pjrt's multi-core path but
    caches the jitted shard_map so repeat kernel() calls skip retracing.
    """
    if "runner" in _NC_CACHE:
        return _NC_CACHE["runner"]

    import jax
    import concourse.mybir as _mb
    from concourse.bass2jax import (_bass_exec_p, install_neuronx_cc_hook,
                                    partition_id_tensor)
    from jax.experimental.shard_map import shard_map
    from jax.sharding import Mesh, PartitionSpec

    nc = _build_nc()
    install_neuronx_cc_hook()

    partition_name = nc.partition_id_tensor.name if nc.partition_id_tensor else None
    in_names, out_names, out_avals, zero_shapes = [], [], [], []
    for alloc in nc.m.functions[0].allocations:
        if not isinstance(alloc, _mb.MemoryLocationSet):
            continue
        name = alloc.memorylocations[0].name
        if alloc.kind == "ExternalInput":
            if name != partition_name:
                in_names.append(name)
        elif alloc.kind == "ExternalOutput":
            shape = tuple(alloc.tensor_shape)
            dtype = _mb.dt.np(alloc.dtype)
            out_names.append(name)
            out_avals.append(jax.core.ShapedArray(shape, dtype))
            zero_shapes.append((shape, dtype))
    n_params = len(in_names)
    all_names = list(in_names) + list(out_names)
    if partition_name is not None:
        all_names.append(partition_name)
    donate = tuple(range(n_params, n_params + len(out_names)))

    def _body(*args):
        operands = list(args)
        if partition_name is not None:
            operands.append(partition_id_tensor())
        outs = _bass_exec_p.bind(
            *operands,
            out_avals=tuple(out_avals),
            in_names=tuple(all_names),
            out_names=tuple(out_names),
            lowering_input_output_aliases=(),
            sim_require_finite=True,
            sim_require_nnan=True,
            nc=nc,
        )
        return tuple(outs)

    devices = jax.devices()[:CORES]
    mesh = Mesh(np.asarray(devices), ("core",))
    nin = n_params + len(out_names)
    sharded = jax.jit(
        shard_map(_body, mesh=mesh, in_specs=(PartitionSpec("core"),) * nin,
                  out_specs=(PartitionSpec("core"),) * len(out_names),
                  check_rep=False),
        donate_argnums=donate, keep_unused=True,
    )

    _NC_CACHE["runner"] = (sharded, in_names, out_names, zero_shapes)
    return _NC_CACHE["runner"]


def kernel(inputs, labels, H, check_weight):
    inputs = np.asarray(inputs, dtype=np.float32)
    labels = np.asarray(labels)
    H = np.asarray(H)
    check_weight = np.asarray(check_weight, dtype=np.float32)

    # host-side constants
    Hf = H.astype(np.float32)                      # [63, 127]
    hf_pad = np.zeros((MH, N), np.float32)
    hf_pad[:M] = Hf
    bigoff = (1.0 - hf_pad) * np.float32(BIGC)     # 1e30 off-support (pad rows all on)
    bigoff[M:] = np.float32(BIGC)
    k_row = hf_pad.sum(axis=1).astype(np.float32)  # row weights (pad rows 0)
    c1 = (k_row * 0.5).astype(np.float32)          # #neg = c1 - S/2
    w = float(check_weight.reshape(-1)[0])
    wprime = np.float32(np.log1p(np.exp(w)))       # softplus(w)

    sharded, in_names, out_names, zero_shapes = _get_runner()

    # per-core inputs, concatenated on axis 0 (one shard per core)
    soft_all = np.ascontiguousarray(
        inputs.reshape(CORES, J, P, N).transpose(0, 2, 1, 3)
    ).reshape(CORES * P, J, N)                      # [8*128, 2, 127]
    per_core_vals = {
        "soft": soft_all,
        "hf": np.tile(hf_pad, (CORES, 1)),
        "bigoff": np.tile(bigoff, (CORES, 1)),
        "c1": np.tile(c1, CORES),
        "wp": np.tile(np.array([wprime], np.float32), CORES),
    }
    args = [per_core_vals[name] for name in in_names]
    zeros = [np.zeros((CORES * s[0], *s[1:]), dt) for s, dt in zero_shapes]
    out_arrs = sharded(*args, *zeros)
    o = np.asarray(out_arrs[out_names.index("out")])  # [8*9, 128, 2, 127]
    o = o.reshape(CORES, ITERS + 1, P, J, N)
    outs = np.ascontiguousarray(
        o.transpose(1, 0, 3, 2, 4)).reshape(ITERS + 1, CORES * BL, N)

    # loss from outs[2] (host side, mirrors reference._loss)
    labels_f = labels.astype(np.float32)
    z = -outs[2]
    ce = np.mean(np.maximum(z, 0.0) - z * labels_f +
                 np.log1p(np.exp(-np.abs(z))), dtype=np.float32)
    prob = 1.0 / (1.0 + np.exp(-z, dtype=np.float32))
    mse = np.sum(np.square(prob - labels_f), dtype=np.float32)
    loss = np.float32(L_WEIGHT * ce + (1.0 - L_WEIGHT) * mse * SCALAR_FACTOR)
    return outs, loss


# revision 12
# speedup vs baseline: 1.0341x; 1.0341x over previous
import sys

if '/opt/trn_rl_repo' not in sys.path:
    sys.path.insert(0, '/opt/trn_rl_repo')

import numpy as np

import concourse.bass as bass
import concourse.bacc as bacc
import concourse.mybir as mybir
from concourse.tile import TileContext
from concourse.bass_utils import run_bass_kernel_spmd

# Problem constants (hardcoded per harness contract)
M, N, ITERS = 63, 127, 8
MH = 64            # m padded to 64 for clean power-of-2 tree sums
P, J = 128, 2      # 128 partitions x 2 batch rows per partition = 256 batch/core
CORES = 8
BL = P * J         # batch per core
CH = 8             # m-chunk size (MH = 8 chunks of 8)
NCHUNK = MH // CH
BIGC = 1e30
L_WEIGHT = 0.5
SCALAR_FACTOR = 0.01

F32 = mybir.dt.float32
BF16 = mybir.dt.bfloat16
ALU = mybir.AluOpType
ACT = mybir.ActivationFunctionType
AX = mybir.AxisListType

_NC_CACHE = {}


def _bcast_dram(ap, parts=P):
    """Partition-broadcast read AP for a DRAM tensor (step 0 over partitions)."""
    return bass.AP(tensor=ap.tensor, offset=0,
                   ap=[[0, parts]] + [list(d) for d in ap.ap])


def _build_nc():
    nc = bacc.Bacc(None, target_bir_lowering=False)

    soft_d = nc.dram_tensor("soft", [P, J, N], F32, kind="ExternalInput")
    hf_d = nc.dram_tensor("hf", [MH, N], F32, kind="ExternalInput")
    bigoff_d = nc.dram_tensor("bigoff", [MH, N], F32, kind="ExternalInput")
    c1_d = nc.dram_tensor("c1", [MH], F32, kind="ExternalInput")
    wp_d = nc.dram_tensor("wp", [1], F32, kind="ExternalInput")
    out_d = nc.dram_tensor("out", [ITERS + 1, P, J, N], F32, kind="ExternalOutput")

    with TileContext(nc) as tc:
        with (
            tc.tile_pool(name="const", bufs=1) as cpool,
            tc.tile_pool(name="state", bufs=1) as spool,
            tc.tile_pool(name="tpool", bufs=2) as tpool,
            tc.tile_pool(name="work", bufs=2) as wpool,
            tc.tile_pool(name="tree", bufs=2) as rpool,
            tc.tile_pool(name="small", bufs=2) as mpool,
        ):
            # ---- constants ----
            hf = cpool.tile([P, MH, N], BF16, tag="hf")
            bigoff = cpool.tile([P, MH, N], BF16, tag="bigoff")
            c1 = cpool.tile([P, MH], F32, tag="c1")
            wp = cpool.tile([P, 1], F32, tag="wp")
            wm8 = cpool.tile([P, 1], F32, tag="wm8")
            soft = cpool.tile([P, J, N], F32, tag="soft")

            nc.gpsimd.dma_start(out=hf, in_=_bcast_dram(hf_d[:]))
            nc.gpsimd.dma_start(out=bigoff, in_=_bcast_dram(bigoff_d[:]))
            nc.gpsimd.dma_start(out=c1, in_=_bcast_dram(c1_d[:]))
            nc.gpsimd.dma_start(out=wp, in_=_bcast_dram(wp_d[:]))
            nc.gpsimd.dma_start(out=soft, in_=soft_d[:])
            nc.vector.tensor_scalar(out=wm8, in0=wp, scalar1=-8.0, scalar2=None,
                                    op0=ALU.mult)

            # ---- state ----
            cv = spool.tile([P, J, MH, N], F32, tag="cv")

            nc.gpsimd.memset(cv[:], 0.0)

            # T(0) = soft;  outs[0] = soft
            T = tpool.tile([P, J, N], F32, tag="T")
            nc.vector.tensor_copy(T[:], soft[:])
            nc.sync.dma_start(out=out_d[:][0], in_=soft)

            for it in range(ITERS):
                Tacc = tpool.tile([P, J, N], F32, tag="Tacc")
                for c in range(NCHUNK):
                    c0, c1e = c * CH, (c + 1) * CH
                    cvc = cv[:, :, c0:c1e, :]
                    T_b = T[:, :, None, :].broadcast_to([P, J, CH, N])
                    bo_v = bigoff[:, None, c0:c1e, :].broadcast_to([P, J, CH, N])
                    hf_v = hf[:, None, c0:c1e, :].broadcast_to([P, J, CH, N])
                    c1_v = c1[:, None, c0:c1e].broadcast_to([P, J, CH])

                    t0 = wpool.tile([P, J, CH, N], F32, tag="A")
                    nc.gpsimd.tensor_tensor(out=t0, in0=T_b, in1=cvc, op=ALU.subtract)
                    vm = wpool.tile([P, J, CH, N], F32, tag="B")
                    nc.vector.scalar_tensor_tensor(out=vm, in0=bo_v, scalar=-1.0,
                                                   in1=t0, op0=ALU.mult, op1=ALU.add)
                    min1c = mpool.tile([P, J, CH], F32, tag="m1")
                    nc.vector.tensor_reduce(out=min1c, in_=vm, axis=AX.X,
                                            op=ALU.min, apply_absolute_value=True)
                    a = wpool.tile([P, J, CH, N], F32, tag="A")
                    nc.scalar.activation(out=a, in_=vm, func=ACT.Abs)
                    smrc = wpool.tile([P, J, CH, N], BF16, tag="C")
                    nc.scalar.activation(out=smrc, in_=vm, func=ACT.Sign)
                    smhc = wpool.tile([P, J, CH, N], BF16, tag="D")
                    nc.gpsimd.tensor_tensor(out=smhc, in0=smrc, in1=hf_v, op=ALU.mult)
                    Sc = mpool.tile([P, J, CH], F32, tag="Sc")
                    nc.vector.tensor_reduce(out=Sc, in_=smhc, axis=AX.X, op=ALU.add)
                    eqc = wpool.tile([P, J, CH, N], mybir.dt.uint32, tag="E")
                    m1_b = min1c[:, :, :, None].broadcast_to([P, J, CH, N])
                    nc.vector.tensor_tensor(out=eqc, in0=a, in1=m1_b, op=ALU.is_equal)
                    k = wpool.tile([P, J, CH, N], F32, tag="B")
                    nc.vector.scalar_tensor_tensor(out=k, in0=eqc, scalar=BIGC, in1=a,
                                                   op0=ALU.mult, op1=ALU.add)
                    min2c = mpool.tile([P, J, CH], F32, tag="m2")
                    nc.vector.tensor_reduce(out=min2c, in_=k, axis=AX.X, op=ALU.min)

                    # parity -> pw = w' * (-1)^(#neg), via round-to-even mod-2
                    nn = mpool.tile([P, J, CH], F32, tag="nn")
                    nc.vector.scalar_tensor_tensor(out=nn, in0=Sc, scalar=-0.5,
                                                   in1=c1_v, op0=ALU.mult, op1=ALU.add)
                    t1r = mpool.tile([P, J, CH], F32, tag="t1r")
                    nc.vector.tensor_scalar(out=t1r, in0=nn, scalar1=0.5,
                                            scalar2=float(2 ** 23),
                                            op0=ALU.mult, op1=ALU.add)
                    t2r = mpool.tile([P, J, CH], F32, tag="t2r")
                    nc.vector.tensor_scalar(out=t2r, in0=t1r, scalar1=float(2 ** 23),
                                            scalar2=None, op0=ALU.subtract)
                    fr = mpool.tile([P, J, CH], F32, tag="fr")
                    nc.vector.scalar_tensor_tensor(out=fr, in0=nn, scalar=0.5,
                                                   in1=t2r, op0=ALU.mult,
                                                   op1=ALU.subtract)
                    p4 = mpool.tile([P, J, CH], F32, tag="p4")
                    nc.vector.tensor_tensor(out=p4, in0=fr, in1=fr, op=ALU.mult)
                    pwc = mpool.tile([P, J, CH], F32, tag="pwc")
                    nc.vector.tensor_scalar(out=pwc, in0=p4, scalar1=wm8, scalar2=wp,
                                            op0=ALU.mult, op1=ALU.add)
                    Pmc = mpool.tile([P, J, CH], F32, tag="Pmc")
                    nc.vector.tensor_tensor(out=Pmc, in0=pwc, in1=min1c, op=ALU.mult)
                    P2mc = mpool.tile([P, J, CH], F32, tag="P2mc")
                    nc.vector.tensor_tensor(out=P2mc, in0=pwc, in1=min2c, op=ALU.mult)

                    # cv_new = (eq ? pw*min2 : pw*min1) * sign * mask
                    Pm_b = Pmc[:, :, :, None].broadcast_to([P, J, CH, N])
                    P2m_b = P2mc[:, :, :, None].broadcast_to([P, J, CH, N])
                    v = wpool.tile([P, J, CH, N], F32, tag="B")
                    nc.scalar.copy(v, Pm_b)
                    nc.vector.copy_predicated(v, eqc, P2m_b)
                    nc.vector.tensor_tensor(out=cv[:, :, c0:c1e, :], in0=v,
                                            in1=smhc, op=ALU.mult)

                    # tree-sum this chunk of new cv into Tacc (GPSIMD)
                    s1 = rpool.tile([P, J, 4, N], F32, tag="s1")
                    nc.gpsimd.tensor_tensor(out=s1, in0=cv[:, :, c0:c0 + 4, :],
                                            in1=cv[:, :, c0 + 4:c1e, :], op=ALU.add)
                    s2 = rpool.tile([P, J, 2, N], F32, tag="s2")
                    nc.gpsimd.tensor_tensor(out=s2, in0=s1[:, :, 0:2, :],
                                            in1=s1[:, :, 2:4, :], op=ALU.add)
                    if c == 0:
                        nc.gpsimd.tensor_tensor(out=Tacc, in0=s2[:, :, 0, :],
                                                in1=s2[:, :, 1, :], op=ALU.add)
                    else:
                        s3 = rpool.tile([P, J, N], F32, tag="s3")
                        nc.gpsimd.tensor_tensor(out=s3, in0=s2[:, :, 0, :],
                                                in1=s2[:, :, 1, :], op=ALU.add)
                        nc.gpsimd.tensor_tensor(out=Tacc, in0=Tacc, in1=s3,
                                                op=ALU.add)

                # T(it+1) = Tacc + soft ; DMA out
                T = tpool.tile([P, J, N], F32, tag="T")
                nc.vector.tensor_tensor(out=T, in0=Tacc, in1=soft, op=ALU.add)
                nc.sync.dma_start(out=out_d[:][it + 1], in_=T)

    nc.finalize()
    return nc


def _get_runner():
    """Build (once) a persistent jitted 8-core runner for the bass kernel.

    Replicates concourse.bass2jax.run_bass_via_pjrt's multi-core path but
    caches the jitted shard_map so repeat kernel() calls skip retracing.
    """
    if "runner" in _NC_CACHE:
        return _NC_CACHE["runner"]

    import jax
    import concourse.mybir as _mb
    from concourse.bass2jax import (_bass_exec_p, install_neuronx_cc_hook,
                                    partition_id_tensor)
    from jax.experimental.shard_map import shard_map
    from jax.sharding import Mesh, PartitionSpec

    nc = _build_nc()
    install_neuronx_cc_hook()

    partition_name = nc.partition_id_tensor.name if nc.partition_id_tensor else None
    in_names, out_names, out_avals, zero_shapes = [], [], [], []
    for alloc in nc.m.functions[0].allocations:
        if not isinstance(alloc, _mb.MemoryLocationSet):
            continue
        name = alloc.memorylocations[0].name
        if alloc.kind == "ExternalInput":
            if name != partition_name:
                in_names.append(name)
        elif alloc.kind == "ExternalOutput":
            shape = tuple(alloc.tensor_shape)
            dtype = _mb.dt.np(alloc.dtype)
            out_names.append(name)
            out_avals.append(jax.core.ShapedArray(shape, dtype))
            zero_shapes.append((shape, dtype))
    n_params = len(in_names)
    all_names = list(in_names) + list(out_names)
    if partition_name is not None:
        all_names.append(partition_name)
    donate = tuple(range(n_params, n_params + len(out_names)))

    def _body(*args):
        operands = list(args)
        if partition_name is not None:
            operands.append(partition_id_tensor())
        outs = _bass_exec_p.bind(
            *operands,
            out_avals=tuple(out_avals),
            in_names=tuple(all_names),
            out_names=tuple(out_names),
            lowering_input_output_aliases=(),
            sim_require_finite=True,
            sim_require_nnan=True,
            nc=nc,
        )
        return tuple(outs)

    devices = jax.devices()[:CORES]
    mesh = Mesh(np.asarray(devices), ("core",))
    nin = n_params + len(out_names)
    sharded = jax.jit(
        shard_map(_body, mesh=mesh, in_specs=(PartitionSpec("core"),) * nin,
                  out_specs=(PartitionSpec("core"),) * len(out_names),
                  check_rep=False),
        donate_argnums=donate, keep_unused=True,
    )

    _NC_CACHE["runner"] = (sharded, in_names, out_names, zero_shapes)
    return _NC_CACHE["runner"]


def kernel(inputs, labels, H, check_weight):
    inputs = np.asarray(inputs, dtype=np.float32)
    labels = np.asarray(labels)
    H = np.asarray(H)
    check_weight = np.asarray(check_weight, dtype=np.float32)

    # host-side constants
    Hf = H.astype(np.float32)                      # [63, 127]
    hf_pad = np.zeros((MH, N), np.float32)
    hf_pad[:M] = Hf
    bigoff = (1.0 - hf_pad) * np.float32(BIGC)     # 1e30 off-support (pad rows all on)
    bigoff[M:] = np.float32(BIGC)
    k_row = hf_pad.sum(axis=1).astype(np.float32)  # row weights (pad rows 0)
    c1 = (k_row * 0.5).astype(np.float32)          # #neg = c1 - S/2
    w = float(check_weight.reshape(-1)[0])
    wprime = np.float32(np.log1p(np.exp(w)))       # softplus(w)

    sharded, in_names, out_names, zero_shapes = _get_runner()

    # per-core inputs, concatenated on axis 0 (one shard per core)
    soft_all = np.ascontiguousarray(
        inputs.reshape(CORES, J, P, N).transpose(0, 2, 1, 3)
    ).reshape(CORES * P, J, N)                      # [8*128, 2, 127]
    per_core_vals = {
        "soft": soft_all,
        "hf": np.tile(hf_pad, (CORES, 1)),
        "bigoff": np.tile(bigoff, (CORES, 1)),
        "c1": np.tile(c1, CORES),
        "wp": np.tile(np.array([wprime], np.float32), CORES),
    }
    args = [per_core_vals[name] for name in in_names]
    zeros = [np.zeros((CORES * s[0], *s[1:]), dt) for s, dt in zero_shapes]
    out_arrs = sharded(*args, *zeros)
    o = np.asarray(out_arrs[out_names.index("out")])  # [8*9, 128, 2, 127]
    o = o.reshape(CORES, ITERS + 1, P, J, N)
    outs = np.ascontiguousarray(
        o.transpose(1, 0, 3, 2, 4)).reshape(ITERS + 1, CORES * BL, N)

    # loss from outs[2] (host side, mirrors reference._loss)
    labels_f = labels.astype(np.float32)
    z = -outs[2]
    ce = np.mean(np.maximum(z, 0.0) - z * labels_f +
                 np.log1p(np.exp(-np.abs(z))), dtype=np.float32)
    prob = 1.0 / (1.0 + np.exp(-z, dtype=np.float32))
    mse = np.sum(np.square(prob - labels_f), dtype=np.float32)
    loss = np.float32(L_WEIGHT * ce + (1.0 - L_WEIGHT) * mse * SCALAR_FACTOR)
    return outs, loss


# revision 13
# speedup vs baseline: 258.3104x; 249.7822x over previous
import sys

if '/opt/trn_rl_repo' not in sys.path:
    sys.path.insert(0, '/opt/trn_rl_repo')

import numpy as np

import concourse.bass as bass
import concourse.bacc as bacc
import concourse.mybir as mybir
from concourse.tile import TileContext
from concourse.bass_utils import run_bass_kernel_spmd

# Problem constants (hardcoded per harness contract)
M, N, ITERS = 63, 127, 8
MH = 64            # m padded to 64 for clean power-of-2 tree sums
P, J = 128, 2      # 128 partitions x 2 batch rows per partition = 256 batch/core
CORES = 8
BL = P * J         # batch per core
CH = 8             # m-chunk size (MH = 8 chunks of 8)
NCHUNK = MH // CH
BIGC = 1e30
L_WEIGHT = 0.5
SCALAR_FACTOR = 0.01

F32 = mybir.dt.float32
BF16 = mybir.dt.bfloat16
ALU = mybir.AluOpType
ACT = mybir.ActivationFunctionType
AX = mybir.AxisListType

_NC_CACHE = {}


def _bcast_dram(ap, parts=P):
    """Partition-broadcast read AP for a DRAM tensor (step 0 over partitions)."""
    return bass.AP(tensor=ap.tensor, offset=0,
                   ap=[[0, parts]] + [list(d) for d in ap.ap])


def _build_nc():
    nc = bacc.Bacc(None, target_bir_lowering=False)

    soft_d = nc.dram_tensor("soft", [P, J, N], F32, kind="ExternalInput")
    hf_d = nc.dram_tensor("hf", [MH, N], F32, kind="ExternalInput")
    bigoff_d = nc.dram_tensor("bigoff", [MH, N], F32, kind="ExternalInput")
    c1_d = nc.dram_tensor("c1", [MH], F32, kind="ExternalInput")
    wp_d = nc.dram_tensor("wp", [1], F32, kind="ExternalInput")
    out_d = nc.dram_tensor("out", [ITERS + 1, P, J, N], F32, kind="ExternalOutput")

    with TileContext(nc) as tc:
        with (
            tc.tile_pool(name="const", bufs=1) as cpool,
            tc.tile_pool(name="state", bufs=1) as spool,
            tc.tile_pool(name="tpool", bufs=2) as tpool,
            tc.tile_pool(name="work", bufs=2) as wpool,
            tc.tile_pool(name="tree", bufs=2) as rpool,
            tc.tile_pool(name="small", bufs=2) as mpool,
        ):
            # ---- constants ----
            hf = cpool.tile([P, MH, N], BF16, tag="hf")
            bigoff = cpool.tile([P, MH, N], BF16, tag="bigoff")
            c1 = cpool.tile([P, MH], F32, tag="c1")
            wp = cpool.tile([P, 1], F32, tag="wp")
            wm8 = cpool.tile([P, 1], F32, tag="wm8")
            soft = cpool.tile([P, J, N], F32, tag="soft")

            nc.gpsimd.dma_start(out=hf, in_=_bcast_dram(hf_d[:]))
            nc.gpsimd.dma_start(out=bigoff, in_=_bcast_dram(bigoff_d[:]))
            nc.gpsimd.dma_start(out=c1, in_=_bcast_dram(c1_d[:]))
            nc.gpsimd.dma_start(out=wp, in_=_bcast_dram(wp_d[:]))
            nc.gpsimd.dma_start(out=soft, in_=soft_d[:])
            nc.vector.tensor_scalar(out=wm8, in0=wp, scalar1=-8.0, scalar2=None,
                                    op0=ALU.mult)

            # ---- state ----
            cv = spool.tile([P, J, MH, N], F32, tag="cv")

            nc.gpsimd.memset(cv[:], 0.0)

            # T(0) = soft;  outs[0] = soft
            T = tpool.tile([P, J, N], F32, tag="T")
            nc.vector.tensor_copy(T[:], soft[:])
            nc.sync.dma_start(out=out_d[:][0], in_=soft)

            for it in range(ITERS):
                Tacc = tpool.tile([P, J, N], F32, tag="Tacc")
                for c in range(NCHUNK):
                    c0, c1e = c * CH, (c + 1) * CH
                    cvc = cv[:, :, c0:c1e, :]
                    T_b = T[:, :, None, :].broadcast_to([P, J, CH, N])
                    bo_v = bigoff[:, None, c0:c1e, :].broadcast_to([P, J, CH, N])
                    hf_v = hf[:, None, c0:c1e, :].broadcast_to([P, J, CH, N])
                    c1_v = c1[:, None, c0:c1e].broadcast_to([P, J, CH])

                    t0 = wpool.tile([P, J, CH, N], F32, tag="A")
                    nc.gpsimd.tensor_tensor(out=t0, in0=T_b, in1=cvc, op=ALU.subtract)
                    vm = wpool.tile([P, J, CH, N], F32, tag="B")
                    nc.vector.scalar_tensor_tensor(out=vm, in0=bo_v, scalar=-1.0,
                                                   in1=t0, op0=ALU.mult, op1=ALU.add)
                    min1c = mpool.tile([P, J, CH], F32, tag="m1")
                    nc.vector.tensor_reduce(out=min1c, in_=vm, axis=AX.X,
                                            op=ALU.min, apply_absolute_value=True)
                    a = wpool.tile([P, J, CH, N], F32, tag="A")
                    nc.scalar.activation(out=a, in_=vm, func=ACT.Abs)
                    smrc = wpool.tile([P, J, CH, N], BF16, tag="C")
                    nc.scalar.activation(out=smrc, in_=vm, func=ACT.Sign)
                    smhc = wpool.tile([P, J, CH, N], BF16, tag="D")
                    nc.gpsimd.tensor_tensor(out=smhc, in0=smrc, in1=hf_v, op=ALU.mult)
                    Sc = mpool.tile([P, J, CH], F32, tag="Sc")
                    nc.vector.tensor_reduce(out=Sc, in_=smhc, axis=AX.X, op=ALU.add)
                    eqc = wpool.tile([P, J, CH, N], mybir.dt.uint32, tag="E")
                    m1_b = min1c[:, :, :, None].broadcast_to([P, J, CH, N])
                    nc.vector.tensor_tensor(out=eqc, in0=a, in1=m1_b, op=ALU.is_equal)
                    k = wpool.tile([P, J, CH, N], F32, tag="B")
                    nc.vector.scalar_tensor_tensor(out=k, in0=eqc, scalar=BIGC, in1=a,
                                                   op0=ALU.mult, op1=ALU.add)
                    min2c = mpool.tile([P, J, CH], F32, tag="m2")
                    nc.vector.tensor_reduce(out=min2c, in_=k, axis=AX.X, op=ALU.min)

                    # parity -> pw = w' * (-1)^(#neg), via round-to-even mod-2
                    nn = mpool.tile([P, J, CH], F32, tag="nn")
                    nc.vector.scalar_tensor_tensor(out=nn, in0=Sc, scalar=-0.5,
                                                   in1=c1_v, op0=ALU.mult, op1=ALU.add)
                    t1r = mpool.tile([P, J, CH], F32, tag="t1r")
                    nc.vector.tensor_scalar(out=t1r, in0=nn, scalar1=0.5,
                                            scalar2=float(2 ** 23),
                                            op0=ALU.mult, op1=ALU.add)
                    t2r = mpool.tile([P, J, CH], F32, tag="t2r")
                    nc.vector.tensor_scalar(out=t2r, in0=t1r, scalar1=float(2 ** 23),
                                            scalar2=None, op0=ALU.subtract)
                    fr = mpool.tile([P, J, CH], F32, tag="fr")
                    nc.vector.scalar_tensor_tensor(out=fr, in0=nn, scalar=0.5,
                                                   in1=t2r, op0=ALU.mult,
                                                   op1=ALU.subtract)
                    p4 = mpool.tile([P, J, CH], F32, tag="p4")
                    nc.vector.tensor_tensor(out=p4, in0=fr, in1=fr, op=ALU.mult)
                    pwc = mpool.tile([P, J, CH], F32, tag="pwc")
                    nc.vector.tensor_scalar(out=pwc, in0=p4, scalar1=wm8, scalar2=wp,
                                            op0=ALU.mult, op1=ALU.add)
                    Pmc = mpool.tile([P, J, CH], F32, tag="Pmc")
                    nc.vector.tensor_tensor(out=Pmc, in0=pwc, in1=min1c, op=ALU.mult)
                    P2mc = mpool.tile([P, J, CH], F32, tag="P2mc")
                    nc.vector.tensor_tensor(out=P2mc, in0=pwc, in1=min2c, op=ALU.mult)

                    # cv_new = (eq ? pw*min2 : pw*min1) * sign * mask
                    Pm_b = Pmc[:, :, :, None].broadcast_to([P, J, CH, N])
                    P2m_b = P2mc[:, :, :, None].broadcast_to([P, J, CH, N])
                    v = wpool.tile([P, J, CH, N], F32, tag="B")
                    nc.scalar.copy(v, Pm_b)
                    nc.vector.copy_predicated(v, eqc, P2m_b)
                    nc.vector.tensor_tensor(out=cv[:, :, c0:c1e, :], in0=v,
                                            in1=smhc, op=ALU.mult)

                    # tree-sum this chunk of new cv into Tacc (GPSIMD)
                    s1 = rpool.tile([P, J, 4, N], F32, tag="s1")
                    nc.gpsimd.tensor_tensor(out=s1, in0=cv[:, :, c0:c0 + 4, :],
                                            in1=cv[:, :, c0 + 4:c1e, :], op=ALU.add)
                    s2 = rpool.tile([P, J, 2, N], F32, tag="s2")
                    nc.gpsimd.tensor_tensor(out=s2, in0=s1[:, :, 0:2, :],
                                            in1=s1[:, :, 2:4, :], op=ALU.add)
                    if c == 0:
                        nc.gpsimd.tensor_tensor(out=Tacc, in0=s2[:, :, 0, :],
                                                in1=s2[:, :, 1, :], op=ALU.add)
                    else:
                        s3 = rpool.tile([P, J, N], F32, tag="s3")
                        nc.gpsimd.tensor_tensor(out=s3, in0=s2[:, :, 0, :],
                                                in1=s2[:, :, 1, :], op=ALU.add)
                        nc.gpsimd.tensor_tensor(out=Tacc, in0=Tacc, in1=s3,
                                                op=ALU.add)

                # T(it+1) = Tacc + soft ; DMA out
                T = tpool.tile([P, J, N], F32, tag="T")
                nc.vector.tensor_tensor(out=T, in0=Tacc, in1=soft, op=ALU.add)
                nc.sync.dma_start(out=out_d[:][it + 1], in_=T)

    nc.finalize()
    return nc


def _get_runner():
    """Build (once) a persistent jitted 8-core runner for the bass kernel.

    Replicates concourse.bass2jax.run_bass_via_pjrt's multi-core path but
    caches the jitted shard_map so repeat kernel() calls skip retracing.
    """
    if "runner" in _NC_CACHE:
        return _NC_CACHE["runner"]

    import jax
    import concourse.mybir as _mb
    from concourse.bass2jax import (_bass_exec_p, install_neuronx_cc_hook,
                                    partition_id_tensor)
    from jax.experimental.shard_map import shard_map
    from jax.sharding import Mesh, PartitionSpec

    nc = _build_nc()
    install_neuronx_cc_hook()

    partition_name = nc.partition_id_tensor.name if nc.partition_id_tensor else None
    in_names, out_names, out_avals, zero_shapes = [], [], [], []
    for alloc in nc.m.functions[0].allocations:
        if not isinstance(alloc, _mb.MemoryLocationSet):
            continue
        name = alloc.memorylocations[0].name
        if alloc.kind == "ExternalInput":
            if name != partition_name:
                in_names.append(name)
        elif alloc.kind == "ExternalOutput":
            shape = tuple(alloc.tensor_shape)
            dtype = _mb.dt.np(alloc.dtype)
            out_names.append(name)
            out_avals.append(jax.core.ShapedArray(shape, dtype))
            zero_shapes.append((shape, dtype))
    n_params = len(in_names)
    all_names = list(in_names) + list(out_names)
    if partition_name is not None:
        all_names.append(partition_name)
    donate = tuple(range(n_params, n_params + len(out_names)))

    def _body(*args):
        operands = list(args)
        if partition_name is not None:
            operands.append(partition_id_tensor())
        outs = _bass_exec_p.bind(
            *operands,
            out_avals=tuple(out_avals),
            in_names=tuple(all_names),
            out_names=tuple(out_names),
            lowering_input_output_aliases=(),
            sim_require_finite=True,
            sim_require_nnan=True,
            nc=nc,
        )
        return tuple(outs)

    devices = jax.devices()[:CORES]
    mesh = Mesh(np.asarray(devices), ("core",))
    nin = n_params + len(out_names)
    sharded = jax.jit(
        shard_map(_body, mesh=mesh, in_specs=(PartitionSpec("core"),) * nin,
                  out_specs=(PartitionSpec("core"),) * len(out_names),
                  check_rep=False),
        donate_argnums=donate, keep_unused=True,
    )

    _NC_CACHE["runner"] = (sharded, in_names, out_names, zero_shapes)
    return _NC_CACHE["runner"]


def kernel(inputs, labels, H, check_weight):
    inputs = np.asarray(inputs, dtype=np.float32)
    labels = np.asarray(labels)
    H = np.asarray(H)
    check_weight = np.asarray(check_weight, dtype=np.float32)

    # host-side constants
    Hf = H.astype(np.float32)                      # [63, 127]
    hf_pad = np.zeros((MH, N), np.float32)
    hf_pad[:M] = Hf
    bigoff = (1.0 - hf_pad) * np.float32(BIGC)     # 1e30 off-support (pad rows all on)
    bigoff[M:] = np.float32(BIGC)
    k_row = hf_pad.sum(axis=1).astype(np.float32)  # row weights (pad rows 0)
    c1 = (k_row * 0.5).astype(np.float32)          # #neg = c1 - S/2
    w = float(check_weight.reshape(-1)[0])
    wprime = np.float32(np.log1p(np.exp(w)))       # softplus(w)

    # per-core inputs, concatenated on axis 0 (one shard per core)
    soft_all = np.ascontiguousarray(
        inputs.reshape(CORES, J, P, N).transpose(0, 2, 1, 3)
    ).reshape(CORES * P, J, N)                      # [8*128, 2, 127]
    per_core_vals = {
        "soft": soft_all,
        "hf": np.tile(hf_pad, (CORES, 1)),
        "bigoff": np.tile(bigoff, (CORES, 1)),
        "c1": np.tile(c1, CORES),
        "wp": np.tile(np.array([wprime], np.float32), CORES),
    }

    try:
        sharded, in_names, out_names, zero_shapes = _get_runner()
        args = [per_core_vals[name] for name in in_names]
        zeros = [np.zeros((CORES * s[0], *s[1:]), dt) for s, dt in zero_shapes]
        out_arrs = sharded(*args, *zeros)
        o = np.asarray(out_arrs[out_names.index("out")])  # [8*9, 128, 2, 127]
        o = o.reshape(CORES, ITERS + 1, P, J, N)
    except Exception:
        # fallback: plain bass_utils path (no jit caching)
        if "nc" not in _NC_CACHE:
            _NC_CACHE["nc"] = _build_nc()
        in_maps = []
        for c in range(CORES):
            m = {k: (v[c * P:(c + 1) * P] if k == "soft"
                     else v[c * MH:(c + 1) * MH] if k in ("hf", "bigoff")
                     else v[c * MH:(c + 1) * MH] if k == "c1"
                     else v[c:c + 1])
                 for k, v in per_core_vals.items()}
            in_maps.append(m)
        res = run_bass_kernel_spmd(_NC_CACHE["nc"], in_maps,
                                   core_ids=list(range(CORES)))
        o = np.stack([res.results[c]["out"] for c in range(CORES)])

    outs = np.ascontiguousarray(
        o.transpose(1, 0, 3, 2, 4)).reshape(ITERS + 1, CORES * BL, N)

    # loss from outs[2] (host side, mirrors reference._loss)
    labels_f = labels.astype(np.float32)
    z = -outs[2]
    ce = np.mean(np.maximum(z, 0.0) - z * labels_f +
                 np.log1p(np.exp(-np.abs(z))), dtype=np.float32)
    prob = 1.0 / (1.0 + np.exp(-z, dtype=np.float32))
    mse = np.sum(np.square(prob - labels_f), dtype=np.float32)
    loss = np.float32(L_WEIGHT * ce + (1.0 - L_WEIGHT) * mse * SCALAR_FACTOR)
    return outs, loss


# revision 19
# speedup vs baseline: 285.2384x; 1.1042x over previous
import sys

if '/opt/trn_rl_repo' not in sys.path:
    sys.path.insert(0, '/opt/trn_rl_repo')

import numpy as np

import concourse.bass as bass
import concourse.bacc as bacc
import concourse.mybir as mybir
from concourse.tile import TileContext
from concourse.bass_utils import run_bass_kernel_spmd

# Problem constants (hardcoded per harness contract)
M, N, ITERS = 63, 127, 8
MH = 64            # m padded to 64 for clean power-of-2 tree sums
P, J = 128, 2      # 128 partitions x 2 batch rows per partition = 256 batch/core
CORES = 8
BL = P * J         # batch per core
CH = 8             # m-chunk size (MH = 8 chunks of 8)
NCHUNK = MH // CH
BIGC = 1e30
L_WEIGHT = 0.5
SCALAR_FACTOR = 0.01

F32 = mybir.dt.float32
BF16 = mybir.dt.bfloat16
ALU = mybir.AluOpType
ACT = mybir.ActivationFunctionType
AX = mybir.AxisListType

_NC_CACHE = {}


def _bcast_dram(ap, parts=P):
    """Partition-broadcast read AP for a DRAM tensor (step 0 over partitions)."""
    return bass.AP(tensor=ap.tensor, offset=0,
                   ap=[[0, parts]] + [list(d) for d in ap.ap])


def _build_nc():
    nc = bacc.Bacc(None, target_bir_lowering=False)

    soft_d = nc.dram_tensor("soft", [P, J, N], F32, kind="ExternalInput")
    hf_d = nc.dram_tensor("hf", [MH, N], F32, kind="ExternalInput")
    bigoff_d = nc.dram_tensor("bigoff", [MH, N], F32, kind="ExternalInput")
    c1_d = nc.dram_tensor("c1", [MH], F32, kind="ExternalInput")
    wp_d = nc.dram_tensor("wp", [1], F32, kind="ExternalInput")
    out_d = nc.dram_tensor("out", [ITERS + 1, P, J, N], F32, kind="ExternalOutput")

    with TileContext(nc) as tc:
        with (
            tc.tile_pool(name="const", bufs=1) as cpool,
            tc.tile_pool(name="state", bufs=1) as spool,
            tc.tile_pool(name="tpool", bufs=2) as tpool,
            tc.tile_pool(name="work", bufs=3) as wpool,
            tc.tile_pool(name="tree", bufs=2) as rpool,
            tc.tile_pool(name="small", bufs=2) as mpool,
        ):
            # ---- constants ----
            hf = cpool.tile([P, MH, N], BF16, tag="hf")
            bigoff = cpool.tile([P, MH, N], BF16, tag="bigoff")
            c1 = cpool.tile([P, MH], F32, tag="c1")
            wp = cpool.tile([P, 1], F32, tag="wp")
            wm8 = cpool.tile([P, 1], F32, tag="wm8")
            soft = cpool.tile([P, J, N], F32, tag="soft")

            nc.gpsimd.dma_start(out=hf, in_=_bcast_dram(hf_d[:]))
            nc.gpsimd.dma_start(out=bigoff, in_=_bcast_dram(bigoff_d[:]))
            nc.gpsimd.dma_start(out=c1, in_=_bcast_dram(c1_d[:]))
            nc.gpsimd.dma_start(out=wp, in_=_bcast_dram(wp_d[:]))
            nc.gpsimd.dma_start(out=soft, in_=soft_d[:])
            nc.vector.tensor_scalar(out=wm8, in0=wp, scalar1=-8.0, scalar2=None,
                                    op0=ALU.mult)

            # ---- state ----
            cv = spool.tile([P, J, MH, N], F32, tag="cv")

            nc.gpsimd.memset(cv[:], 0.0)

            # T(0) = soft;  outs[0] = soft
            T = tpool.tile([P, J, N], F32, tag="T")
            nc.vector.tensor_copy(T[:], soft[:])
            nc.sync.dma_start(out=out_d[:][0], in_=soft)

            for it in range(ITERS):
                Tacc = tpool.tile([P, J, N], F32, tag="Tacc")
                for c in range(NCHUNK):
                    c0, c1e = c * CH, (c + 1) * CH
                    cvc = cv[:, :, c0:c1e, :]
                    T_b = T[:, :, None, :].broadcast_to([P, J, CH, N])
                    bo_v = bigoff[:, None, c0:c1e, :].broadcast_to([P, J, CH, N])
                    hf_v = hf[:, None, c0:c1e, :].broadcast_to([P, J, CH, N])
                    c1_v = c1[:, None, c0:c1e].broadcast_to([P, J, CH])

                    t0 = wpool.tile([P, J, CH, N], F32, tag="A")
                    nc.gpsimd.tensor_tensor(out=t0, in0=T_b, in1=cvc, op=ALU.subtract)
                    vm = wpool.tile([P, J, CH, N], F32, tag="B")
                    nc.gpsimd.tensor_tensor(out=vm, in0=t0, in1=bo_v,
                                            op=ALU.subtract)
                    min1c = mpool.tile([P, J, CH], F32, tag="m1")
                    nc.vector.tensor_reduce(out=min1c, in_=vm, axis=AX.X,
                                            op=ALU.min, apply_absolute_value=True)
                    a = wpool.tile([P, J, CH, N], F32, tag="A")
                    nc.scalar.activation(out=a, in_=vm, func=ACT.Abs)
                    smrc = wpool.tile([P, J, CH, N], BF16, tag="C")
                    nc.scalar.activation(out=smrc, in_=vm, func=ACT.Sign)
                    smhc = wpool.tile([P, J, CH, N], BF16, tag="D")
                    nc.gpsimd.tensor_tensor(out=smhc, in0=smrc, in1=hf_v, op=ALU.mult)
                    Sc = mpool.tile([P, J, CH], F32, tag="Sc")
                    nc.vector.tensor_reduce(out=Sc, in_=smhc, axis=AX.X, op=ALU.add)
                    eqc = wpool.tile([P, J, CH, N], mybir.dt.uint8, tag="E")
                    m1_b = min1c[:, :, :, None].broadcast_to([P, J, CH, N])
                    nc.vector.tensor_tensor(out=eqc, in0=a, in1=m1_b, op=ALU.is_equal)
                    k = wpool.tile([P, J, CH, N], F32, tag="B")
                    nc.vector.scalar_tensor_tensor(out=k, in0=eqc, scalar=BIGC, in1=a,
                                                   op0=ALU.mult, op1=ALU.add)
                    min2c = mpool.tile([P, J, CH], F32, tag="m2")
                    nc.vector.tensor_reduce(out=min2c, in_=k, axis=AX.X, op=ALU.min)

                    # parity -> pw = w' * (-1)^(#neg), via round-to-even mod-2
                    nn = mpool.tile([P, J, CH], F32, tag="nn")
                    nc.vector.scalar_tensor_tensor(out=nn, in0=Sc, scalar=-0.5,
                                                   in1=c1_v, op0=ALU.mult, op1=ALU.add)
                    t1r = mpool.tile([P, J, CH], F32, tag="t1r")
                    nc.vector.tensor_scalar(out=t1r, in0=nn, scalar1=0.5,
                                            scalar2=float(2 ** 23),
                                            op0=ALU.mult, op1=ALU.add)
                    t2r = mpool.tile([P, J, CH], F32, tag="t2r")
                    nc.vector.tensor_scalar(out=t2r, in0=t1r, scalar1=float(2 ** 23),
                                            scalar2=None, op0=ALU.subtract)
                    fr = mpool.tile([P, J, CH], F32, tag="fr")
                    nc.vector.scalar_tensor_tensor(out=fr, in0=nn, scalar=0.5,
                                                   in1=t2r, op0=ALU.mult,
                                                   op1=ALU.subtract)
                    p4 = mpool.tile([P, J, CH], F32, tag="p4")
                    nc.vector.tensor_tensor(out=p4, in0=fr, in1=fr, op=ALU.mult)
                    pwc = mpool.tile([P, J, CH], F32, tag="pwc")
                    nc.vector.tensor_scalar(out=pwc, in0=p4, scalar1=wm8, scalar2=wp,
                                            op0=ALU.mult, op1=ALU.add)
                    Pmc = mpool.tile([P, J, CH], F32, tag="Pmc")
                    nc.vector.tensor_tensor(out=Pmc, in0=pwc, in1=min1c, op=ALU.mult)
                    P2mc = mpool.tile([P, J, CH], F32, tag="P2mc")
                    nc.vector.tensor_tensor(out=P2mc, in0=pwc, in1=min2c, op=ALU.mult)

                    # cv_new = (eq ? pw*min2 : pw*min1) * sign * mask
                    Pm_b = Pmc[:, :, :, None].broadcast_to([P, J, CH, N])
                    P2m_b = P2mc[:, :, :, None].broadcast_to([P, J, CH, N])
                    v = wpool.tile([P, J, CH, N], F32, tag="B")
                    nc.scalar.copy(v, Pm_b)
                    nc.vector.copy_predicated(v, eqc, P2m_b)
                    nc.vector.tensor_tensor(out=cv[:, :, c0:c1e, :], in0=v,
                                            in1=smhc, op=ALU.mult)

                    # tree-sum this chunk of new cv into Tacc (GPSIMD)
                    s1 = rpool.tile([P, J, 4, N], F32, tag="s1")
                    nc.gpsimd.tensor_tensor(out=s1, in0=cv[:, :, c0:c0 + 4, :],
                                            in1=cv[:, :, c0 + 4:c1e, :], op=ALU.add)
                    s2 = rpool.tile([P, J, 2, N], F32, tag="s2")
                    nc.gpsimd.tensor_tensor(out=s2, in0=s1[:, :, 0:2, :],
                                            in1=s1[:, :, 2:4, :], op=ALU.add)
                    if c == 0:
                        nc.gpsimd.tensor_tensor(out=Tacc, in0=s2[:, :, 0, :],
                                                in1=s2[:, :, 1, :], op=ALU.add)
                    else:
                        s3 = rpool.tile([P, J, N], F32, tag="s3")
                        nc.gpsimd.tensor_tensor(out=s3, in0=s2[:, :, 0, :],
                                                in1=s2[:, :, 1, :], op=ALU.add)
                        nc.gpsimd.tensor_tensor(out=Tacc, in0=Tacc, in1=s3,
                                                op=ALU.add)

                # T(it+1) = Tacc + soft ; DMA out
                T = tpool.tile([P, J, N], F32, tag="T")
                nc.vector.tensor_tensor(out=T, in0=Tacc, in1=soft, op=ALU.add)
                nc.sync.dma_start(out=out_d[:][it + 1], in_=T)

    nc.finalize()
    return nc


def _get_runner():
    """Build (once) a persistent jitted 8-core runner for the bass kernel.

    Replicates concourse.bass2jax.run_bass_via_pjrt's multi-core path but
    caches the jitted shard_map so repeat kernel() calls skip retracing.
    """
    if "runner" in _NC_CACHE:
        return _NC_CACHE["runner"]

    import jax
    import concourse.mybir as _mb
    from concourse.bass2jax import (_bass_exec_p, install_neuronx_cc_hook,
                                    partition_id_tensor)
    from jax.experimental.shard_map import shard_map
    from jax.sharding import Mesh, PartitionSpec

    nc = _build_nc()
    install_neuronx_cc_hook()

    partition_name = nc.partition_id_tensor.name if nc.partition_id_tensor else None
    in_names, out_names, out_avals, zero_shapes = [], [], [], []
    for alloc in nc.m.functions[0].allocations:
        if not isinstance(alloc, _mb.MemoryLocationSet):
            continue
        name = alloc.memorylocations[0].name
        if alloc.kind == "ExternalInput":
            if name != partition_name:
                in_names.append(name)
        elif alloc.kind == "ExternalOutput":
            shape = tuple(alloc.tensor_shape)
            dtype = _mb.dt.np(alloc.dtype)
            out_names.append(name)
            out_avals.append(jax.core.ShapedArray(shape, dtype))
            zero_shapes.append((shape, dtype))
    n_params = len(in_names)
    all_names = list(in_names) + list(out_names)
    if partition_name is not None:
        all_names.append(partition_name)
    donate = tuple(range(n_params, n_params + len(out_names)))

    def _body(*args):
        operands = list(args)
        if partition_name is not None:
            operands.append(partition_id_tensor())
        outs = _bass_exec_p.bind(
            *operands,
            out_avals=tuple(out_avals),
            in_names=tuple(all_names),
            out_names=tuple(out_names),
            lowering_input_output_aliases=(),
            sim_require_finite=True,
            sim_require_nnan=True,
            nc=nc,
        )
        return tuple(outs)

    devices = jax.devices()[:CORES]
    mesh = Mesh(np.asarray(devices), ("core",))
    nin = n_params + len(out_names)
    sharded = jax.jit(
        shard_map(_body, mesh=mesh, in_specs=(PartitionSpec("core"),) * nin,
                  out_specs=(PartitionSpec("core"),) * len(out_names),
                  check_rep=False),
        donate_argnums=donate, keep_unused=True,
    )

    _NC_CACHE["runner"] = (sharded, in_names, out_names, zero_shapes)
    return _NC_CACHE["runner"]


def kernel(inputs, labels, H, check_weight):
    inputs = np.asarray(inputs, dtype=np.float32)
    labels = np.asarray(labels)
    H = np.asarray(H)
    check_weight = np.asarray(check_weight, dtype=np.float32)

    # host-side constants
    Hf = H.astype(np.float32)                      # [63, 127]
    hf_pad = np.zeros((MH, N), np.float32)
    hf_pad[:M] = Hf
    bigoff = (1.0 - hf_pad) * np.float32(BIGC)     # 1e30 off-support (pad rows all on)
    bigoff[M:] = np.float32(BIGC)
    k_row = hf_pad.sum(axis=1).astype(np.float32)  # row weights (pad rows 0)
    c1 = (k_row * 0.5).astype(np.float32)          # #neg = c1 - S/2
    w = float(check_weight.reshape(-1)[0])
    wprime = np.float32(np.log1p(np.exp(w)))       # softplus(w)

    # per-core inputs, concatenated on axis 0 (one shard per core)
    soft_all = np.ascontiguousarray(
        inputs.reshape(CORES, J, P, N).transpose(0, 2, 1, 3)
    ).reshape(CORES * P, J, N)                      # [8*128, 2, 127]
    per_core_vals = {
        "soft": soft_all,
        "hf": np.tile(hf_pad, (CORES, 1)),
        "bigoff": np.tile(bigoff, (CORES, 1)),
        "c1": np.tile(c1, CORES),
        "wp": np.tile(np.array([wprime], np.float32), CORES),
    }

    try:
        sharded, in_names, out_names, zero_shapes = _get_runner()
        args = [per_core_vals[name] for name in in_names]
        zeros = [np.zeros((CORES * s[0], *s[1:]), dt) for s, dt in zero_shapes]
        out_arrs = sharded(*args, *zeros)
        o = np.asarray(out_arrs[out_names.index("out")])  # [8*9, 128, 2, 127]
        o = o.reshape(CORES, ITERS + 1, P, J, N)
    except Exception:
        # fallback: plain bass_utils path (no jit caching)
        if "nc" not in _NC_CACHE:
            _NC_CACHE["nc"] = _build_nc()
        in_maps = []
        for c in range(CORES):
            m = {k: (v[c * P:(c + 1) * P] if k == "soft"
                     else v[c * MH:(c + 1) * MH] if k in ("hf", "bigoff")
                     else v[c * MH:(c + 1) * MH] if k == "c1"
                     else v[c:c + 1])
                 for k, v in per_core_vals.items()}
            in_maps.append(m)
        res = run_bass_kernel_spmd(_NC_CACHE["nc"], in_maps,
                                   core_ids=list(range(CORES)))
        o = np.stack([res.results[c]["out"] for c in range(CORES)])

    outs = np.ascontiguousarray(
        o.transpose(1, 0, 3, 2, 4)).reshape(ITERS + 1, CORES * BL, N)

    # loss from outs[2] (host side, mirrors reference._loss)
    labels_f = labels.astype(np.float32)
    z = -outs[2]
    ce = np.mean(np.maximum(z, 0.0) - z * labels_f +
                 np.log1p(np.exp(-np.abs(z))), dtype=np.float32)
    prob = 1.0 / (1.0 + np.exp(-z, dtype=np.float32))
    mse = np.sum(np.square(prob - labels_f), dtype=np.float32)
    loss = np.float32(L_WEIGHT * ce + (1.0 - L_WEIGHT) * mse * SCALAR_FACTOR)
    return outs, loss


# revision 23
# speedup vs baseline: 287.6653x; 1.0085x over previous
import sys

if '/opt/trn_rl_repo' not in sys.path:
    sys.path.insert(0, '/opt/trn_rl_repo')

import numpy as np

import concourse.bass as bass
import concourse.bacc as bacc
import concourse.mybir as mybir
from concourse.tile import TileContext
from concourse.bass_utils import run_bass_kernel_spmd

# Problem constants (hardcoded per harness contract)
M, N, ITERS = 63, 127, 8
MH = 64            # m padded to 64 for clean power-of-2 tree sums
P, J = 128, 2      # 128 partitions x 2 batch rows per partition = 256 batch/core
CORES = 8
BL = P * J         # batch per core
CH = 8             # m-chunk size (MH = 8 chunks of 8)
NCHUNK = MH // CH
BIGC = 1e30
L_WEIGHT = 0.5
SCALAR_FACTOR = 0.01

F32 = mybir.dt.float32
BF16 = mybir.dt.bfloat16
ALU = mybir.AluOpType
ACT = mybir.ActivationFunctionType
AX = mybir.AxisListType

_NC_CACHE = {}


def _bcast_dram(ap, parts=P):
    """Partition-broadcast read AP for a DRAM tensor (step 0 over partitions)."""
    return bass.AP(tensor=ap.tensor, offset=0,
                   ap=[[0, parts]] + [list(d) for d in ap.ap])


def _build_nc():
    nc = bacc.Bacc(None, target_bir_lowering=False)

    soft_d = nc.dram_tensor("soft", [P, J, N], F32, kind="ExternalInput")
    hf_d = nc.dram_tensor("hf", [MH, N], F32, kind="ExternalInput")
    bigoff_d = nc.dram_tensor("bigoff", [MH, N], F32, kind="ExternalInput")
    c1_d = nc.dram_tensor("c1", [MH], F32, kind="ExternalInput")
    wp_d = nc.dram_tensor("wp", [1], F32, kind="ExternalInput")
    out_d = nc.dram_tensor("out", [ITERS + 1, P, J, N], F32, kind="ExternalOutput")

    with TileContext(nc) as tc:
        with (
            tc.tile_pool(name="const", bufs=1) as cpool,
            tc.tile_pool(name="state", bufs=1) as spool,
            tc.tile_pool(name="tpool", bufs=2) as tpool,
            tc.tile_pool(name="work", bufs=3) as wpool,
            tc.tile_pool(name="tree", bufs=2) as rpool,
            tc.tile_pool(name="small", bufs=2) as mpool,
        ):
            # ---- constants ----
            hf = cpool.tile([P, MH, N], BF16, tag="hf")
            bigoff = cpool.tile([P, MH, N], BF16, tag="bigoff")
            c1 = cpool.tile([P, MH], F32, tag="c1")
            wp = cpool.tile([P, 1], F32, tag="wp")
            wm8 = cpool.tile([P, 1], F32, tag="wm8")
            soft = cpool.tile([P, J, N], F32, tag="soft")

            nc.gpsimd.dma_start(out=hf, in_=_bcast_dram(hf_d[:]))
            nc.gpsimd.dma_start(out=bigoff, in_=_bcast_dram(bigoff_d[:]))
            nc.gpsimd.dma_start(out=c1, in_=_bcast_dram(c1_d[:]))
            nc.gpsimd.dma_start(out=wp, in_=_bcast_dram(wp_d[:]))
            nc.gpsimd.dma_start(out=soft, in_=soft_d[:])
            nc.vector.tensor_scalar(out=wm8, in0=wp, scalar1=-8.0, scalar2=None,
                                    op0=ALU.mult)

            # ---- state ----
            cv = spool.tile([P, J, MH, N], F32, tag="cv")

            nc.gpsimd.memset(cv[:], 0.0)

            # T(0) = soft;  outs[0] = soft
            T = tpool.tile([P, J, N], F32, tag="T")
            nc.vector.tensor_copy(T[:], soft[:])
            nc.sync.dma_start(out=out_d[:][0], in_=soft)

            for it in range(ITERS):
                Tacc = tpool.tile([P, J, N], F32, tag="Tacc")
                for c in range(NCHUNK):
                    c0, c1e = c * CH, (c + 1) * CH
                    cvc = cv[:, :, c0:c1e, :]
                    T_b = T[:, :, None, :].broadcast_to([P, J, CH, N])
                    bo_v = bigoff[:, None, c0:c1e, :].broadcast_to([P, J, CH, N])
                    hf_v = hf[:, None, c0:c1e, :].broadcast_to([P, J, CH, N])
                    c1_v = c1[:, None, c0:c1e].broadcast_to([P, J, CH])

                    vm = wpool.tile([P, J, CH, N], F32, tag="B")
                    if it == 0:
                        nc.gpsimd.tensor_tensor(out=vm, in0=T_b, in1=bo_v,
                                                op=ALU.subtract)
                    else:
                        t0 = wpool.tile([P, J, CH, N], F32, tag="A")
                        nc.gpsimd.tensor_tensor(out=t0, in0=T_b, in1=cvc,
                                                op=ALU.subtract)
                        nc.gpsimd.tensor_tensor(out=vm, in0=t0, in1=bo_v,
                                                op=ALU.subtract)
                    min1c = mpool.tile([P, J, CH], F32, tag="m1")
                    nc.vector.tensor_reduce(out=min1c, in_=vm, axis=AX.X,
                                            op=ALU.min, apply_absolute_value=True)
                    a = wpool.tile([P, J, CH, N], F32, tag="A")
                    nc.scalar.activation(out=a, in_=vm, func=ACT.Abs)
                    smrc = wpool.tile([P, J, CH, N], BF16, tag="C")
                    nc.scalar.activation(out=smrc, in_=vm, func=ACT.Sign)
                    smhc = wpool.tile([P, J, CH, N], BF16, tag="D")
                    nc.gpsimd.tensor_tensor(out=smhc, in0=smrc, in1=hf_v, op=ALU.mult)
                    Sc = mpool.tile([P, J, CH], F32, tag="Sc")
                    nc.vector.tensor_reduce(out=Sc, in_=smrc, axis=AX.X, op=ALU.add)
                    eqc = wpool.tile([P, J, CH, N], mybir.dt.uint8, tag="E")
                    m1_b = min1c[:, :, :, None].broadcast_to([P, J, CH, N])
                    nc.vector.tensor_tensor(out=eqc, in0=a, in1=m1_b, op=ALU.is_equal)
                    k = wpool.tile([P, J, CH, N], F32, tag="B")
                    nc.vector.scalar_tensor_tensor(out=k, in0=eqc, scalar=BIGC, in1=a,
                                                   op0=ALU.mult, op1=ALU.add)
                    min2c = mpool.tile([P, J, CH], F32, tag="m2")
                    nc.vector.tensor_reduce(out=min2c, in_=k, axis=AX.X, op=ALU.min)

                    # parity -> pw = w' * (-1)^(#neg), via round-to-even mod-2
                    nn = mpool.tile([P, J, CH], F32, tag="nn")
                    nc.vector.scalar_tensor_tensor(out=nn, in0=Sc, scalar=-0.5,
                                                   in1=c1_v, op0=ALU.mult, op1=ALU.add)
                    t1r = mpool.tile([P, J, CH], F32, tag="t1r")
                    nc.vector.tensor_scalar(out=t1r, in0=nn, scalar1=0.5,
                                            scalar2=float(2 ** 23),
                                            op0=ALU.mult, op1=ALU.add)
                    t2r = mpool.tile([P, J, CH], F32, tag="t2r")
                    nc.vector.tensor_scalar(out=t2r, in0=t1r, scalar1=float(2 ** 23),
                                            scalar2=None, op0=ALU.subtract)
                    fr = mpool.tile([P, J, CH], F32, tag="fr")
                    nc.vector.scalar_tensor_tensor(out=fr, in0=nn, scalar=0.5,
                                                   in1=t2r, op0=ALU.mult,
                                                   op1=ALU.subtract)
                    p4 = mpool.tile([P, J, CH], F32, tag="p4")
                    nc.vector.tensor_tensor(out=p4, in0=fr, in1=fr, op=ALU.mult)
                    pwc = mpool.tile([P, J, CH], F32, tag="pwc")
                    nc.vector.tensor_scalar(out=pwc, in0=p4, scalar1=wm8, scalar2=wp,
                                            op0=ALU.mult, op1=ALU.add)
                    Pmc = mpool.tile([P, J, CH], F32, tag="Pmc")
                    nc.vector.tensor_tensor(out=Pmc, in0=pwc, in1=min1c, op=ALU.mult)
                    P2mc = mpool.tile([P, J, CH], F32, tag="P2mc")
                    nc.vector.tensor_tensor(out=P2mc, in0=pwc, in1=min2c, op=ALU.mult)

                    # cv_new = (eq ? pw*min2 : pw*min1) * sign * mask
                    Pm_b = Pmc[:, :, :, None].broadcast_to([P, J, CH, N])
                    P2m_b = P2mc[:, :, :, None].broadcast_to([P, J, CH, N])
                    v = wpool.tile([P, J, CH, N], F32, tag="B")
                    nc.scalar.copy(v, Pm_b)
                    nc.vector.copy_predicated(v, eqc, P2m_b)
                    nc.vector.tensor_tensor(out=cv[:, :, c0:c1e, :], in0=v,
                                            in1=smhc, op=ALU.mult)

                    # tree-sum this chunk of new cv into Tacc (GPSIMD)
                    s1 = rpool.tile([P, J, 4, N], F32, tag="s1")
                    nc.gpsimd.tensor_tensor(out=s1, in0=cv[:, :, c0:c0 + 4, :],
                                            in1=cv[:, :, c0 + 4:c1e, :], op=ALU.add)
                    s2 = rpool.tile([P, J, 2, N], F32, tag="s2")
                    nc.gpsimd.tensor_tensor(out=s2, in0=s1[:, :, 0:2, :],
                                            in1=s1[:, :, 2:4, :], op=ALU.add)
                    if c == 0:
                        nc.gpsimd.tensor_tensor(out=Tacc, in0=s2[:, :, 0, :],
                                                in1=s2[:, :, 1, :], op=ALU.add)
                    else:
                        s3 = rpool.tile([P, J, N], F32, tag="s3")
                        nc.gpsimd.tensor_tensor(out=s3, in0=s2[:, :, 0, :],
                                                in1=s2[:, :, 1, :], op=ALU.add)
                        nc.gpsimd.tensor_tensor(out=Tacc, in0=Tacc, in1=s3,
                                                op=ALU.add)

                # T(it+1) = Tacc + soft ; DMA out
                T = tpool.tile([P, J, N], F32, tag="T")
                nc.vector.tensor_tensor(out=T, in0=Tacc, in1=soft, op=ALU.add)
                nc.sync.dma_start(out=out_d[:][it + 1], in_=T)

    nc.finalize()
    return nc


def _get_runner():
    """Build (once) a persistent jitted 8-core runner for the bass kernel.

    Replicates concourse.bass2jax.run_bass_via_pjrt's multi-core path but
    caches the jitted shard_map so repeat kernel() calls skip retracing.
    """
    if "runner" in _NC_CACHE:
        return _NC_CACHE["runner"]

    import jax
    import concourse.mybir as _mb
    from concourse.bass2jax import (_bass_exec_p, install_neuronx_cc_hook,
                                    partition_id_tensor)
    from jax.experimental.shard_map import shard_map
    from jax.sharding import Mesh, PartitionSpec

    nc = _build_nc()
    install_neuronx_cc_hook()

    partition_name = nc.partition_id_tensor.name if nc.partition_id_tensor else None
    in_names, out_names, out_avals, zero_shapes = [], [], [], []
    for alloc in nc.m.functions[0].allocations:
        if not isinstance(alloc, _mb.MemoryLocationSet):
            continue
        name = alloc.memorylocations[0].name
        if alloc.kind == "ExternalInput":
            if name != partition_name:
                in_names.append(name)
        elif alloc.kind == "ExternalOutput":
            shape = tuple(alloc.tensor_shape)
            dtype = _mb.dt.np(alloc.dtype)
            out_names.append(name)
            out_avals.append(jax.core.ShapedArray(shape, dtype))
            zero_shapes.append((shape, dtype))
    n_params = len(in_names)
    all_names = list(in_names) + list(out_names)
    if partition_name is not None:
        all_names.append(partition_name)
    donate = tuple(range(n_params, n_params + len(out_names)))

    def _body(*args):
        operands = list(args)
        if partition_name is not None:
            operands.append(partition_id_tensor())
        outs = _bass_exec_p.bind(
            *operands,
            out_avals=tuple(out_avals),
            in_names=tuple(all_names),
            out_names=tuple(out_names),
            lowering_input_output_aliases=(),
            sim_require_finite=True,
            sim_require_nnan=True,
            nc=nc,
        )
        return tuple(outs)

    devices = jax.devices()[:CORES]
    mesh = Mesh(np.asarray(devices), ("core",))
    nin = n_params + len(out_names)
    sharded = jax.jit(
        shard_map(_body, mesh=mesh, in_specs=(PartitionSpec("core"),) * nin,
                  out_specs=(PartitionSpec("core"),) * len(out_names),
                  check_rep=False),
        donate_argnums=donate, keep_unused=True,
    )

    _NC_CACHE["runner"] = (sharded, in_names, out_names, zero_shapes)
    return _NC_CACHE["runner"]


def kernel(inputs, labels, H, check_weight):
    inputs = np.asarray(inputs, dtype=np.float32)
    labels = np.asarray(labels)
    H = np.asarray(H)
    check_weight = np.asarray(check_weight, dtype=np.float32)

    # host-side constants
    Hf = H.astype(np.float32)                      # [63, 127]
    hf_pad = np.zeros((MH, N), np.float32)
    hf_pad[:M] = Hf
    bigoff = (1.0 - hf_pad) * np.float32(BIGC)     # 1e30 off-support (pad rows all on)
    bigoff[M:] = np.float32(BIGC)
    k_row = hf_pad.sum(axis=1).astype(np.float32)  # row weights (pad rows 0)
    c1 = (k_row - (N - k_row) * 0.5 - k_row * 0.5).astype(np.float32)  # = k - 63.5; #neg = c1 - S'/2
    w = float(check_weight.reshape(-1)[0])
    wprime = np.float32(np.log1p(np.exp(w)))       # softplus(w)

    # per-core inputs, concatenated on axis 0 (one shard per core)
    soft_all = np.ascontiguousarray(
        inputs.reshape(CORES, J, P, N).transpose(0, 2, 1, 3)
    ).reshape(CORES * P, J, N)                      # [8*128, 2, 127]
    per_core_vals = {
        "soft": soft_all,
        "hf": np.tile(hf_pad, (CORES, 1)),
        "bigoff": np.tile(bigoff, (CORES, 1)),
        "c1": np.tile(c1, CORES),
        "wp": np.tile(np.array([wprime], np.float32), CORES),
    }

    try:
        sharded, in_names, out_names, zero_shapes = _get_runner()
        args = [per_core_vals[name] for name in in_names]
        zeros = [np.zeros((CORES * s[0], *s[1:]), dt) for s, dt in zero_shapes]
        out_arrs = sharded(*args, *zeros)
        o = np.asarray(out_arrs[out_names.index("out")])  # [8*9, 128, 2, 127]
        o = o.reshape(CORES, ITERS + 1, P, J, N)
    except Exception:
        # fallback: plain bass_utils path (no jit caching)
        if "nc" not in _NC_CACHE:
            _NC_CACHE["nc"] = _build_nc()
        in_maps = []
        for c in range(CORES):
            m = {k: (v[c * P:(c + 1) * P] if k == "soft"
                     else v[c * MH:(c + 1) * MH] if k in ("hf", "bigoff")
                     else v[c * MH:(c + 1) * MH] if k == "c1"
                     else v[c:c + 1])
                 for k, v in per_core_vals.items()}
            in_maps.append(m)
        res = run_bass_kernel_spmd(_NC_CACHE["nc"], in_maps,
                                   core_ids=list(range(CORES)))
        o = np.stack([res.results[c]["out"] for c in range(CORES)])

    outs = np.ascontiguousarray(
        o.transpose(1, 0, 3, 2, 4)).reshape(ITERS + 1, CORES * BL, N)

    # loss from outs[2] (host side, mirrors reference._loss)
    labels_f = labels.astype(np.float32)
    z = -outs[2]
    ce = np.mean(np.maximum(z, 0.0) - z * labels_f +
                 np.log1p(np.exp(-np.abs(z))), dtype=np.float32)
    prob = 1.0 / (1.0 + np.exp(-z, dtype=np.float32))
    mse = np.sum(np.square(prob - labels_f), dtype=np.float32)
    loss = np.float32(L_WEIGHT * ce + (1.0 - L_WEIGHT) * mse * SCALAR_FACTOR)
    return outs, loss
